# revision 2
# baseline (speedup 1.0000x reference)
"""BiologicalSplatAttentionLayer Trainium2 kernel (8-core SPMD).

Math (per batch b):
    aff[s,k]  = normalize_k( exp(-max(|x_s - c_k|^2, 0) / (2 sig_k^2)) )
    out       = aff @ ((aff.T @ x) @ Wv.T @ Wo.T)
The factored form is algebraically identical to the reference
(values/splat_states associativity through the rank-K bottleneck) and turns
two SxDxD matmuls into KxDxD ones.

Sharding: 8 cores = 4 batches x 2 token-halves (2048 tokens/core). The only
cross-core coupling is y = aff.T @ x ([K, D] per batch), reduced with an
on-device AllReduce over core pairs. Weights/centers are replicated.

Host-side prep is layout only: slicing, transposes, bf16 casts, identity
matrices. All arithmetic (affinities, normalization, matmuls) runs on-device.
"""

import numpy as np
import ml_dtypes

import concourse.bass as bass
import concourse.tile as tile
import concourse.mybir as mybir
from concourse import bacc
from concourse import bass_utils

BF16 = mybir.dt.bfloat16
F32 = mybir.dt.float32
NPBF16 = ml_dtypes.bfloat16

B, S, D, K = 4, 4096, 1024, 64
NCORES = 8
SH = S // 2            # tokens per core
NCH = SH // 128        # 128-token chunks per core (16)
NB = SH // 512         # 512-token blocks per core (4)
ND = D // 128          # contraction chunks (8)

_CACHE = {}


def _build_nc():
    nc = bacc.Bacc("TRN2", debug=False, enable_asserts=False, num_devices=NCORES)

    xn_d = nc.dram_tensor("xn", [SH, D], BF16, kind="ExternalInput")
    xt_d = nc.dram_tensor("xt", [D, SH], BF16, kind="ExternalInput")
    ctb_d = nc.dram_tensor("ctb", [D, K], BF16, kind="ExternalInput")   # (2*centers).T
    cen_d = nc.dram_tensor("cen", [K, D], F32, kind="ExternalInput")
    lsc_d = nc.dram_tensor("lsc", [K, 1], F32, kind="ExternalInput")
    wvt_d = nc.dram_tensor("wvt", [D, D], BF16, kind="ExternalInput")   # Wv.T
    wot_d = nc.dram_tensor("wot", [D, D], BF16, kind="ExternalInput")   # Wo.T
    idb_d = nc.dram_tensor("idb", [128, 128], BF16, kind="ExternalInput")
    idf_d = nc.dram_tensor("idf", [64, 64], F32, kind="ExternalInput")
    ones_d = nc.dram_tensor("ones", [1, 128], F32, kind="ExternalInput")
    out_d = nc.dram_tensor("out", [SH, D], F32, kind="ExternalOutput")

    with tile.TileContext(nc) as tc:
        with (
            tc.tile_pool(name="const", bufs=1) as cpool,
            tc.tile_pool(name="xns", bufs=3) as xn_pool,
            tc.tile_pool(name="scr", bufs=2) as scr_pool,
            tc.tile_pool(name="adj", bufs=2) as adj_pool,
            tc.tile_pool(name="tsb", bufs=3) as t_pool,
            tc.tile_pool(name="osb", bufs=3) as o_pool,
            tc.tile_pool(name="dram", bufs=1, space="DRAM") as dram_pool,
        ):
            # ---- resident tensors -------------------------------------------------
            xt_sb = cpool.tile([128, ND, SH], BF16)
            nc.sync.dma_start(xt_sb[:], xt_d.ap().rearrange("(c p) s -> p c s", p=128))
            ctb_sb = cpool.tile([128, ND, K], BF16)
            nc.sync.dma_start(ctb_sb[:], ctb_d.ap().rearrange("(c p) k -> p c k", p=128))
            wvt_sb = cpool.tile([128, ND, D], BF16)
            nc.sync.dma_start(wvt_sb[:], wvt_d.ap().rearrange("(c p) e -> p c e", p=128))
            wot_sb = cpool.tile([128, ND, D], BF16)
            nc.sync.dma_start(wot_sb[:], wot_d.ap().rearrange("(c p) f -> p c f", p=128))
            cen_sb = cpool.tile([K, D], F32)
            nc.sync.dma_start(cen_sb[:], cen_d.ap())
            lsc_sb = cpool.tile([K, 1], F32)
            nc.sync.dma_start(lsc_sb[:], lsc_d.ap())
            idb_sb = cpool.tile([128, 128], BF16)
            nc.sync.dma_start(idb_sb[:], idb_d.ap())
            idf_sb = cpool.tile([64, 64], F32)
            nc.sync.dma_start(idf_sb[:], idf_d.ap())
            ones_sb = cpool.tile([1, 128], F32)
            nc.sync.dma_start(ones_sb[:], ones_d.ap())

            x2_sb = cpool.tile([128, NCH], F32)      # |x_s|^2 per chunk column
            den_sb = cpool.tile([128, NCH], F32)
            rden_sb = cpool.tile([128, NCH], F32)
            afft_sb = cpool.tile([64, SH], BF16)     # aff.T, chunk-major columns
            y_sb = cpool.tile([K, D], F32)
            yg_sb = cpool.tile([K, D], F32)
            w2_sb = cpool.tile([K, D], F32)
            z_sb = cpool.tile([K, D], BF16)
            yt_sb = cpool.tile([128, ND, K], BF16)
            w2t_sb = cpool.tile([128, ND, K], BF16)

            # ---- splat constants --------------------------------------------------
            sig = cpool.tile([K, 1], F32)
            nc.scalar.activation(sig[:], lsc_sb[:], mybir.ActivationFunctionType.Exp)
            nc.vector.tensor_scalar_max(sig[:], sig[:], 0.1)
            nc.vector.tensor_scalar_min(sig[:], sig[:], 2.0)
            sig2 = cpool.tile([K, 1], F32)
            nc.vector.tensor_tensor(sig2[:], sig[:], sig[:], mybir.AluOpType.mult)
            inv_col = cpool.tile([K, 1], F32)        # 1/(2 sig^2)
            nc.vector.reciprocal(inv_col[:], sig2[:])
            nc.vector.tensor_scalar_mul(inv_col[:], inv_col[:], 0.5)

            c2scr = cpool.tile([K, D], F32)
            c2_col = cpool.tile([K, 1], F32)
            nc.scalar.activation(
                c2scr[:], cen_sb[:], mybir.ActivationFunctionType.Square,
                accum_out=c2_col[:],
            )

            with tc.tile_pool(name="pscst", bufs=2, space="PSUM") as pscst:
                invrow_ps = pscst.tile([1, 64], F32)
                nc.tensor.transpose(invrow_ps[:], inv_col[:], idf_sb[:])
                invrow_sb = cpool.tile([1, 64], F32)
                nc.vector.tensor_copy(invrow_sb[:], invrow_ps[:])
                invb_ps = pscst.tile([128, 64], F32)
                nc.tensor.matmul(invb_ps[:], ones_sb[:], invrow_sb[:])
                invb_sb = cpool.tile([128, 64], F32)
                nc.vector.tensor_copy(invb_sb[:], invb_ps[:])

            # ---- phase 1: affinities + y = aff.T @ x ------------------------------
            with (
                tc.tile_pool(name="psxc", bufs=2, space="PSUM") as psxc,
                tc.tile_pool(name="psbt", bufs=2, space="PSUM") as psbt,
                tc.tile_pool(name="psat", bufs=2, space="PSUM") as psat,
                tc.tile_pool(name="psy", bufs=1, space="PSUM") as psy,
                tc.tile_pool(name="affp", bufs=3) as aff_pool,
            ):
                psum_y = psy.tile([K, D], F32)
                for blk in range(NB):
                    psum_xc = psxc.tile([K, 512], F32)
                    for dj in range(ND):
                        nc.tensor.matmul(
                            psum_xc[:],
                            ctb_sb[:, dj, :],
                            xt_sb[:, dj, blk * 512:(blk + 1) * 512],
                            start=(dj == 0), stop=(dj == ND - 1),
                        )
                    # adj = (2xc - c2) * inv  (k-major layout)
                    adj_sb = adj_pool.tile([K, 512], F32)
                    nc.vector.tensor_scalar(
                        adj_sb[:], psum_xc[:], c2_col[:], inv_col[:],
                        mybir.AluOpType.subtract, mybir.AluOpType.mult,
                    )
                    for j2 in range(4):
                        j = blk * 4 + j2
                        xn_t = xn_pool.tile([128, D], BF16)
                        nc.sync.dma_start(xn_t[:], xn_d.ap()[j * 128:(j + 1) * 128, :])
                        sq = scr_pool.tile([128, D], F32, tag="sq")
                        nc.scalar.activation(
                            sq[:], xn_t[:], mybir.ActivationFunctionType.Square,
                            accum_out=x2_sb[:, j:j + 1],
                        )
                        # transpose adj chunk -> [s, k]
                        bt_ps = psbt.tile([128, 64], F32)
                        nc.tensor.transpose(
                            bt_ps[:], adj_sb[:, j2 * 128:(j2 + 1) * 128], idf_sb[:]
                        )
                        # t = inv*x2 - inv*adj = inv * d2
                        t_sb = t_pool.tile([128, 64], F32, tag="t")
                        nc.vector.scalar_tensor_tensor(
                            t_sb[:], invb_sb[:], x2_sb[:, j:j + 1], bt_ps[:],
                            mybir.AluOpType.mult, mybir.AluOpType.subtract,
                        )
                        nc.vector.tensor_scalar_max(t_sb[:], t_sb[:], 0.0)
                        affu = t_pool.tile([128, 64], F32, tag="affu")
                        nc.scalar.activation(
                            affu[:], t_sb[:], mybir.ActivationFunctionType.Exp,
                            scale=-1.0, accum_out=den_sb[:, j:j + 1],
                        )
                        nc.vector.tensor_scalar_add(
                            den_sb[:, j:j + 1], den_sb[:, j:j + 1], 1e-8
                        )
                        nc.vector.reciprocal(rden_sb[:, j:j + 1], den_sb[:, j:j + 1])
                        aff_bf = aff_pool.tile([128, 64], BF16)
                        nc.vector.tensor_scalar_mul(
                            aff_bf[:], affu[:], rden_sb[:, j:j + 1]
                        )
                        # y += aff.T @ x  (accumulated over all chunks)
                        for dh in range(2):
                            nc.tensor.matmul(
                                psum_y[:, dh * 512:(dh + 1) * 512],
                                aff_bf[:],
                                xn_t[:, dh * 512:(dh + 1) * 512],
                                start=(j == 0), stop=(j == NCH - 1),
                            )
                        # aff.T for the output matmul
                        at_ps = psat.tile([64, 128], BF16)
                        nc.tensor.transpose(at_ps[:], aff_bf[:], idb_sb[:])
                        nc.vector.tensor_copy(
                            afft_sb[:, j * 128:(j + 1) * 128], at_ps[:]
                        )
                nc.vector.tensor_copy(y_sb[:], psum_y[:])

            # ---- phase 1.5: AllReduce y across the batch pair ---------------------
            y_in = dram_pool.tile([K, D], F32)
            y_out = dram_pool.tile([K, D], F32)
            nc.sync.dma_start(y_in[:], y_sb[:])
            nc.gpsimd.collective_compute(
                "AllReduce",
                mybir.AluOpType.add,
                replica_groups=[[0, 1], [2, 3], [4, 5], [6, 7]],
                ins=[y_in.opt()],
                outs=[y_out.opt()],
            )
            nc.sync.dma_start(yg_sb[:], y_out[:])

            # ---- phase 2: Z = (y @ Wv.T) @ Wo.T -----------------------------------
            with (
                tc.tile_pool(name="pst2", bufs=2, space="PSUM") as pst2,
                tc.tile_pool(name="psw", bufs=2, space="PSUM") as psw,
            ):
                for dj in range(ND):
                    tr = pst2.tile([128, 64], F32)
                    nc.tensor.transpose(
                        tr[:], yg_sb[:, dj * 128:(dj + 1) * 128], idf_sb[:]
                    )
                    nc.vector.tensor_copy(yt_sb[:, dj, :], tr[:])
                for eh in range(2):
                    psum_w2 = psw.tile([K, 512], F32, tag="w2")
                    for dj in range(ND):
                        nc.tensor.matmul(
                            psum_w2[:],
                            yt_sb[:, dj, :],
                            wvt_sb[:, dj, eh * 512:(eh + 1) * 512],
                            start=(dj == 0), stop=(dj == ND - 1),
                        )
                    nc.vector.tensor_copy(w2_sb[:, eh * 512:(eh + 1) * 512], psum_w2[:])
                for ej in range(ND):
                    tr = pst2.tile([128, 64], F32)
                    nc.tensor.transpose(
                        tr[:], w2_sb[:, ej * 128:(ej + 1) * 128], idf_sb[:]
                    )
                    nc.vector.tensor_copy(w2t_sb[:, ej, :], tr[:])
                for fh in range(2):
                    psum_z = psw.tile([K, 512], F32, tag="z")
                    for ej in range(ND):
                        nc.tensor.matmul(
                            psum_z[:],
                            w2t_sb[:, ej, :],
                            wot_sb[:, ej, fh * 512:(fh + 1) * 512],
                            start=(ej == 0), stop=(ej == ND - 1),
                        )
                    nc.vector.tensor_copy(z_sb[:, fh * 512:(fh + 1) * 512], psum_z[:])

            # ---- phase 3: out = aff @ Z -------------------------------------------
            with tc.tile_pool(name="pso", bufs=2, space="PSUM") as pso:
                for j in range(NCH):
                    psum_o = pso.tile([128, D], F32)
                    for fh in range(2):
                        nc.tensor.matmul(
                            psum_o[:, fh * 512:(fh + 1) * 512],
                            afft_sb[:, j * 128:(j + 1) * 128],
                            z_sb[:, fh * 512:(fh + 1) * 512],
                            start=True, stop=True,
                        )
                    o_sb = o_pool.tile([128, D], F32)
                    nc.vector.tensor_copy(o_sb[:], psum_o[:])
                    nc.sync.dma_start(out_d.ap()[j * 128:(j + 1) * 128, :], o_sb[:])

    nc.compile()
    return nc


def _get_nc():
    if "nc" not in _CACHE:
        _CACHE["nc"] = _build_nc()
    return _CACHE["nc"]


def kernel(token_embeddings, splat_centers, splat_log_scales, Wv, Wo):
    x = np.asarray(token_embeddings, dtype=np.float32)
    centers = np.asarray(splat_centers, dtype=np.float32)
    log_scales = np.asarray(splat_log_scales, dtype=np.float32)
    Wv = np.asarray(Wv, dtype=np.float32)
    Wo = np.asarray(Wo, dtype=np.float32)

    nc = _get_nc()

    shared = {
        "ctb": np.ascontiguousarray((2.0 * centers).T).astype(NPBF16),
        "cen": centers,
        "lsc": log_scales.reshape(K, 1),
        "wvt": np.ascontiguousarray(Wv.T).astype(NPBF16),
        "wot": np.ascontiguousarray(Wo.T).astype(NPBF16),
        "idb": np.eye(128, dtype=NPBF16),
        "idf": np.eye(64, dtype=np.float32),
        "ones": np.ones((1, 128), dtype=np.float32),
    }
    in_maps = []
    for c in range(NCORES):
        b, h = divmod(c, 2)
        xs = x[b, h * SH:(h + 1) * SH]
        m = dict(shared)
        m["xn"] = xs.astype(NPBF16)
        m["xt"] = np.ascontiguousarray(xs.T).astype(NPBF16)
        in_maps.append(m)

    res = bass_utils.run_bass_kernel_spmd(nc, in_maps, core_ids=list(range(NCORES)))

    out = np.empty((B, S, D), dtype=np.float32)
    for c in range(NCORES):
        b, h = divmod(c, 2)
        out[b, h * SH:(h + 1) * SH] = res.results[c]["out"]
    return out


# revision 3
# speedup vs baseline: 1.0799x; 1.0799x over previous
"""BiologicalSplatAttentionLayer Trainium2 kernel (8-core SPMD).

Math (per batch b):
    aff[s,k]  = normalize_k( exp(-max(|x_s - c_k|^2, 0) / (2 sig_k^2)) )
    out       = aff @ ((aff.T @ x) @ Wv.T @ Wo.T)
The factored form is algebraically identical to the reference
(values/splat_states associativity through the rank-K bottleneck) and turns
two SxDxD matmuls into KxDxD ones.

Sharding: 8 cores = 4 batches x 2 token-halves (2048 tokens/core). The only
cross-core coupling is y = aff.T @ x ([K, D] per batch). Since
Z = (y @ Wv.T) @ Wo.T is linear in y, each core computes Z from its partial y
and a single AllReduce over core pairs sums Z directly - the collective sits
between the small phase-2 matmuls and the output phase.

Host-side prep is layout only: slicing, transposes, bf16 casts, identity
matrices. All arithmetic (affinities, normalization, matmuls) runs on-device.
"""

import numpy as np
import ml_dtypes

import concourse.bass as bass
import concourse.tile as tile
import concourse.mybir as mybir
from concourse import bacc
from concourse import bass_utils

BF16 = mybir.dt.bfloat16
F32 = mybir.dt.float32
NPBF16 = ml_dtypes.bfloat16

B, S, D, K = 4, 4096, 1024, 64
NCORES = 8
SH = S // 2            # tokens per core
NCH = SH // 128        # 128-token chunks per core (16)
NB = SH // 512         # 512-token blocks per core (4)
ND = D // 128          # contraction chunks (8)

OUT_BF16 = True        # store output in bf16 (halves output DMA); host upcasts

_CACHE = {}


def _build_nc():
    nc = bacc.Bacc("TRN2", debug=False, enable_asserts=False, num_devices=NCORES)

    out_dt = BF16 if OUT_BF16 else F32
    xn_d = nc.dram_tensor("xn", [SH, D], BF16, kind="ExternalInput")
    xt_d = nc.dram_tensor("xt", [D, SH], BF16, kind="ExternalInput")
    ctb_d = nc.dram_tensor("ctb", [D, K], BF16, kind="ExternalInput")   # (2*centers).T
    cen_d = nc.dram_tensor("cen", [K, D], F32, kind="ExternalInput")
    lsc_d = nc.dram_tensor("lsc", [K, 1], F32, kind="ExternalInput")
    wvt_d = nc.dram_tensor("wvt", [D, D], BF16, kind="ExternalInput")   # Wv.T
    wot_d = nc.dram_tensor("wot", [D, D], BF16, kind="ExternalInput")   # Wo.T
    idb_d = nc.dram_tensor("idb", [128, 128], BF16, kind="ExternalInput")
    idf_d = nc.dram_tensor("idf", [64, 64], F32, kind="ExternalInput")
    ones_d = nc.dram_tensor("ones", [1, 128], F32, kind="ExternalInput")
    out_d = nc.dram_tensor("out", [SH, D], out_dt, kind="ExternalOutput")

    with tile.TileContext(nc) as tc:
        with (
            tc.tile_pool(name="const", bufs=1) as cpool,
            tc.tile_pool(name="xns", bufs=2) as xn_pool,
            tc.tile_pool(name="scr", bufs=2) as scr_pool,
            tc.tile_pool(name="adj", bufs=2) as adj_pool,
            tc.tile_pool(name="tsb", bufs=4) as t_pool,
            tc.tile_pool(name="osb", bufs=2) as o_pool,
            tc.tile_pool(name="dram", bufs=1, space="DRAM") as dram_pool,
        ):
            # ---- resident tensors. DMA order matters: everything phase-1 needs
            # goes first on the sync HWDGE FIFO; the big projection weights (only
            # needed in phase 2) go on the scalar HWDGE FIFO in parallel.
            lsc_sb = cpool.tile([K, 1], F32)
            nc.sync.dma_start(lsc_sb[:], lsc_d.ap())
            cen_sb = cpool.tile([K, D], F32)
            nc.sync.dma_start(cen_sb[:], cen_d.ap())
            idb_sb = cpool.tile([128, 128], BF16)
            nc.sync.dma_start(idb_sb[:], idb_d.ap())
            idf_sb = cpool.tile([64, 64], F32)
            nc.sync.dma_start(idf_sb[:], idf_d.ap())
            ones_sb = cpool.tile([1, 128], F32)
            nc.sync.dma_start(ones_sb[:], ones_d.ap())
            ctb_sb = cpool.tile([128, ND, K], BF16)
            nc.sync.dma_start(ctb_sb[:], ctb_d.ap().rearrange("(c p) k -> p c k", p=128))
            xt_sb = cpool.tile([128, ND, SH], BF16)
            nc.sync.dma_start(xt_sb[:], xt_d.ap().rearrange("(c p) s -> p c s", p=128))

            wvt_sb = cpool.tile([128, ND, D], BF16)
            nc.scalar.dma_start(wvt_sb[:], wvt_d.ap().rearrange("(c p) e -> p c e", p=128))
            wot_sb = cpool.tile([128, ND, D], BF16)
            nc.scalar.dma_start(wot_sb[:], wot_d.ap().rearrange("(c p) f -> p c f", p=128))

            x2_sb = cpool.tile([128, NCH], F32)      # |x_s|^2 per chunk column
            den_sb = cpool.tile([128, NCH], F32)
            rden_sb = cpool.tile([128, NCH], F32)
            afft_sb = cpool.tile([64, SH], BF16)     # aff.T, chunk-major columns
            y_sb = cpool.tile([K, D], F32)
            z_sb = cpool.tile([K, D], F32)
            zg_sb = cpool.tile([K, D], F32)
            z_bf = cpool.tile([K, D], BF16)
            w2_sb = cpool.tile([K, D], F32)
            yt_sb = cpool.tile([128, ND, K], BF16)
            w2t_sb = cpool.tile([128, ND, K], BF16)

            # ---- splat constants --------------------------------------------------
            sig = cpool.tile([K, 1], F32)
            nc.scalar.activation(sig[:], lsc_sb[:], mybir.ActivationFunctionType.Exp)
            nc.vector.tensor_scalar_max(sig[:], sig[:], 0.1)
            nc.vector.tensor_scalar_min(sig[:], sig[:], 2.0)
            sig2 = cpool.tile([K, 1], F32)
            nc.vector.tensor_tensor(sig2[:], sig[:], sig[:], mybir.AluOpType.mult)
            inv_col = cpool.tile([K, 1], F32)        # 1/(2 sig^2)
            nc.vector.reciprocal(inv_col[:], sig2[:])
            nc.vector.tensor_scalar_mul(inv_col[:], inv_col[:], 0.5)

            c2scr = cpool.tile([K, D], F32)
            c2_col = cpool.tile([K, 1], F32)
            nc.scalar.activation(
                c2scr[:], cen_sb[:], mybir.ActivationFunctionType.Square,
                accum_out=c2_col[:],
            )

            with tc.tile_pool(name="pscst", bufs=2, space="PSUM") as pscst:
                invrow_ps = pscst.tile([1, 64], F32)
                nc.tensor.transpose(invrow_ps[:], inv_col[:], idf_sb[:])
                invrow_sb = cpool.tile([1, 64], F32)
                nc.vector.tensor_copy(invrow_sb[:], invrow_ps[:])
                invb_ps = pscst.tile([128, 64], F32)
                nc.tensor.matmul(invb_ps[:], ones_sb[:], invrow_sb[:])
                invb_sb = cpool.tile([128, 64], F32)
                nc.vector.tensor_copy(invb_sb[:], invb_ps[:])

            # ---- phase 1: affinities + y = aff.T @ x ------------------------------
            with (
                tc.tile_pool(name="psxc", bufs=2, space="PSUM") as psxc,
                tc.tile_pool(name="psbt", bufs=2, space="PSUM") as psbt,
                tc.tile_pool(name="psat", bufs=2, space="PSUM") as psat,
                tc.tile_pool(name="psy", bufs=1, space="PSUM") as psy,
                tc.tile_pool(name="affp", bufs=4) as aff_pool,
            ):
                psum_y = psy.tile([K, D], F32)
                for blk in range(NB):
                    xn_t = xn_pool.tile([128, 4, D], BF16)
                    nc.sync.dma_start(
                        xn_t[:],
                        xn_d.ap()[blk * 512:(blk + 1) * 512, :]
                        .rearrange("(c p) d -> p c d", p=128),
                    )
                    psum_xc = psxc.tile([K, 512], F32)
                    for dj in range(ND):
                        nc.tensor.matmul(
                            psum_xc[:],
                            ctb_sb[:, dj, :],
                            xt_sb[:, dj, blk * 512:(blk + 1) * 512],
                            start=(dj == 0), stop=(dj == ND - 1),
                        )
                    # adj = (2xc - c2) * inv  (k-major layout)
                    adj_sb = adj_pool.tile([K, 512], F32)
                    nc.vector.tensor_scalar(
                        adj_sb[:], psum_xc[:], c2_col[:], inv_col[:],
                        mybir.AluOpType.subtract, mybir.AluOpType.mult,
                    )
                    for j2 in range(4):
                        j = blk * 4 + j2
                        sq = scr_pool.tile([128, D], F32, tag="sq")
                        nc.scalar.activation(
                            sq[:], xn_t[:, j2, :], mybir.ActivationFunctionType.Square,
                            accum_out=x2_sb[:, j:j + 1],
                        )
                        # transpose adj chunk -> [s, k]
                        bt_ps = psbt.tile([128, 64], F32)
                        nc.tensor.transpose(
                            bt_ps[:], adj_sb[:, j2 * 128:(j2 + 1) * 128], idf_sb[:]
                        )
                        # t = inv*x2 - inv*adj = inv * d2
                        t_sb = t_pool.tile([128, 64], F32, tag="t")
                        nc.vector.scalar_tensor_tensor(
                            t_sb[:], invb_sb[:], x2_sb[:, j:j + 1], bt_ps[:],
                            mybir.AluOpType.mult, mybir.AluOpType.subtract,
                        )
                        nc.vector.tensor_scalar_max(t_sb[:], t_sb[:], 0.0)
                        affu = t_pool.tile([128, 64], F32, tag="affu")
                        nc.scalar.activation(
                            affu[:], t_sb[:], mybir.ActivationFunctionType.Exp,
                            scale=-1.0, accum_out=den_sb[:, j:j + 1],
                        )
                        nc.vector.tensor_scalar_add(
                            den_sb[:, j:j + 1], den_sb[:, j:j + 1], 1e-8
                        )
                        nc.vector.reciprocal(rden_sb[:, j:j + 1], den_sb[:, j:j + 1])
                        aff_bf = aff_pool.tile([128, 64], BF16)
                        nc.vector.tensor_scalar_mul(
                            aff_bf[:], affu[:], rden_sb[:, j:j + 1]
                        )
                        # y += aff.T @ x  (accumulated over all chunks)
                        for dh in range(2):
                            nc.tensor.matmul(
                                psum_y[:, dh * 512:(dh + 1) * 512],
                                aff_bf[:],
                                xn_t[:, j2, dh * 512:(dh + 1) * 512],
                                start=(j == 0), stop=(j == NCH - 1),
                            )
                        # aff.T for the output matmul
                        at_ps = psat.tile([64, 128], BF16)
                        nc.tensor.transpose(at_ps[:], aff_bf[:], idb_sb[:])
                        nc.vector.tensor_copy(
                            afft_sb[:, j * 128:(j + 1) * 128], at_ps[:]
                        )
                nc.vector.tensor_copy(y_sb[:], psum_y[:])

            # ---- phase 2: Z_loc = (y_loc @ Wv.T) @ Wo.T ---------------------------
            with (
                tc.tile_pool(name="pst2", bufs=2, space="PSUM") as pst2,
                tc.tile_pool(name="psw", bufs=2, space="PSUM") as psw,
            ):
                for dj in range(ND):
                    tr = pst2.tile([128, 64], F32)
                    nc.tensor.transpose(
                        tr[:], y_sb[:, dj * 128:(dj + 1) * 128], idf_sb[:]
                    )
                    nc.vector.tensor_copy(yt_sb[:, dj, :], tr[:])
                for eh in range(2):
                    psum_w2 = psw.tile([K, 512], F32, tag="w2")
                    for dj in range(ND):
                        nc.tensor.matmul(
                            psum_w2[:],
                            yt_sb[:, dj, :],
                            wvt_sb[:, dj, eh * 512:(eh + 1) * 512],
                            start=(dj == 0), stop=(dj == ND - 1),
                        )
                    nc.vector.tensor_copy(w2_sb[:, eh * 512:(eh + 1) * 512], psum_w2[:])
                for ej in range(ND):
                    tr = pst2.tile([128, 64], F32)
                    nc.tensor.transpose(
                        tr[:], w2_sb[:, ej * 128:(ej + 1) * 128], idf_sb[:]
                    )
                    nc.vector.tensor_copy(w2t_sb[:, ej, :], tr[:])
                for fh in range(2):
                    psum_z = psw.tile([K, 512], F32, tag="z")
                    for ej in range(ND):
                        nc.tensor.matmul(
                            psum_z[:],
                            w2t_sb[:, ej, :],
                            wot_sb[:, ej, fh * 512:(fh + 1) * 512],
                            start=(ej == 0), stop=(ej == ND - 1),
                        )
                    nc.vector.tensor_copy(z_sb[:, fh * 512:(fh + 1) * 512], psum_z[:])

            # ---- phase 2.5: AllReduce Z across the batch pair (Z linear in y) -----
            z_in = dram_pool.tile([K, D], F32)
            z_out = dram_pool.tile([K, D], F32)
            nc.sync.dma_start(z_in[:], z_sb[:])
            nc.gpsimd.collective_compute(
                "AllReduce",
                mybir.AluOpType.add,
                replica_groups=[[0, 1], [2, 3], [4, 5], [6, 7]],
                ins=[z_in.opt()],
                outs=[z_out.opt()],
            )
            nc.sync.dma_start(zg_sb[:], z_out[:])
            nc.vector.tensor_copy(z_bf[:], zg_sb[:])

            # ---- phase 3: out = aff @ Z -------------------------------------------
            with tc.tile_pool(name="pso", bufs=2, space="PSUM") as pso:
                for g in range(NCH // 4):
                    o_sb = o_pool.tile([128, 4, D], out_dt)
                    for j2 in range(4):
                        j = g * 4 + j2
                        psum_o = pso.tile([128, D], F32)
                        for fh in range(2):
                            nc.tensor.matmul(
                                psum_o[:, fh * 512:(fh + 1) * 512],
                                afft_sb[:, j * 128:(j + 1) * 128],
                                z_bf[:, fh * 512:(fh + 1) * 512],
                                start=True, stop=True,
                            )
                        nc.vector.tensor_copy(o_sb[:, j2, :], psum_o[:])
                    nc.sync.dma_start(
                        out_d.ap()[g * 512:(g + 1) * 512, :]
                        .rearrange("(c p) d -> p c d", p=128),
                        o_sb[:],
                    )

    nc.compile()
    return nc


def _get_nc():
    if "nc" not in _CACHE:
        _CACHE["nc"] = _build_nc()
    return _CACHE["nc"]


def kernel(token_embeddings, splat_centers, splat_log_scales, Wv, Wo):
    x = np.asarray(token_embeddings, dtype=np.float32)
    centers = np.asarray(splat_centers, dtype=np.float32)
    log_scales = np.asarray(splat_log_scales, dtype=np.float32)
    Wv = np.asarray(Wv, dtype=np.float32)
    Wo = np.asarray(Wo, dtype=np.float32)

    nc = _get_nc()

    shared = {
        "ctb": np.ascontiguousarray((2.0 * centers).T).astype(NPBF16),
        "cen": centers,
        "lsc": log_scales.reshape(K, 1),
        "wvt": np.ascontiguousarray(Wv.T).astype(NPBF16),
        "wot": np.ascontiguousarray(Wo.T).astype(NPBF16),
        "idb": np.eye(128, dtype=NPBF16),
        "idf": np.eye(64, dtype=np.float32),
        "ones": np.ones((1, 128), dtype=np.float32),
    }
    in_maps = []
    for c in range(NCORES):
        b, h = divmod(c, 2)
        xs = x[b, h * SH:(h + 1) * SH]
        m = dict(shared)
        m["xn"] = xs.astype(NPBF16)
        m["xt"] = np.ascontiguousarray(xs.T).astype(NPBF16)
        in_maps.append(m)

    res = bass_utils.run_bass_kernel_spmd(nc, in_maps, core_ids=list(range(NCORES)))

    out = np.empty((B, S, D), dtype=np.float32)
    for c in range(NCORES):
        b, h = divmod(c, 2)
        out[b, h * SH:(h + 1) * SH] = res.results[c]["out"].astype(np.float32)
    return out


# revision 4
# speedup vs baseline: 1.2218x; 1.1314x over previous
"""BiologicalSplatAttentionLayer Trainium2 kernel (8-core SPMD).

Math (per batch b):
    aff[s,k]  = normalize_k( exp(-max(|x_s - c_k|^2, 0) / (2 sig_k^2)) )
    out       = aff @ ((aff.T @ x) @ Wv.T @ Wo.T)
The factored form is algebraically identical to the reference
(values/splat_states associativity through the rank-K bottleneck) and turns
two SxDxD matmuls into KxDxD ones.

Sharding: 8 cores = 4 batches x 2 token-halves. y = aff.T @ x couples all
tokens of a batch; on-device collectives cost ~45us fixed here, so instead
each core redundantly processes its full batch (streamed in bf16) for the
affinity/aggregation phase and computes only its own token-half of the
output. Each core's token stream is reordered (own half first) host-side so
the SPMD program always outputs chunks 0..15.

Host-side prep is layout only: slicing, transposes, bf16 casts, identity
matrices. All arithmetic (affinities, normalization, matmuls) runs on-device.
"""

import numpy as np
import ml_dtypes

import concourse.bass as bass
import concourse.tile as tile
import concourse.mybir as mybir
from concourse import bacc
from concourse import bass_utils

BF16 = mybir.dt.bfloat16
F32 = mybir.dt.float32
NPBF16 = ml_dtypes.bfloat16

B, S, D, K = 4, 4096, 1024, 64
NCORES = 8
SH = S // 2            # output tokens per core
NCH = S // 128         # processed 128-token chunks per core (32)
NOCH = SH // 128       # output chunks per core (16)
NB = S // 512          # processed 512-token blocks per core (8)
ND = D // 128          # contraction chunks (8)

OUT_BF16 = True        # store output in bf16 (halves output DMA); host upcasts

_CACHE = {}


def _build_nc():
    nc = bacc.Bacc("TRN2", debug=False, enable_asserts=False, num_devices=NCORES)

    out_dt = BF16 if OUT_BF16 else F32
    xn_d = nc.dram_tensor("xn", [S, D], BF16, kind="ExternalInput")
    xt_d = nc.dram_tensor("xt", [D, S], BF16, kind="ExternalInput")
    ctb_d = nc.dram_tensor("ctb", [D, K], BF16, kind="ExternalInput")   # (2*centers).T
    cen_d = nc.dram_tensor("cen", [K, D], F32, kind="ExternalInput")
    lsc_d = nc.dram_tensor("lsc", [K, 1], F32, kind="ExternalInput")
    wvt_d = nc.dram_tensor("wvt", [D, D], BF16, kind="ExternalInput")   # Wv.T
    wot_d = nc.dram_tensor("wot", [D, D], BF16, kind="ExternalInput")   # Wo.T
    idb_d = nc.dram_tensor("idb", [128, 128], BF16, kind="ExternalInput")
    idf_d = nc.dram_tensor("idf", [64, 64], F32, kind="ExternalInput")
    ones_d = nc.dram_tensor("ones", [1, 128], F32, kind="ExternalInput")
    out_d = nc.dram_tensor("out", [SH, D], out_dt, kind="ExternalOutput")

    with tile.TileContext(nc) as tc:
        with (
            tc.tile_pool(name="const", bufs=1) as cpool,
            tc.tile_pool(name="xts", bufs=3) as xt_pool,
            tc.tile_pool(name="xns", bufs=3) as xn_pool,
            tc.tile_pool(name="scr", bufs=2) as scr_pool,
            tc.tile_pool(name="adj", bufs=2) as adj_pool,
            tc.tile_pool(name="tsb", bufs=4) as t_pool,
            tc.tile_pool(name="osb", bufs=2) as o_pool,
        ):
            # ---- small constants first on the sync HWDGE FIFO ---------------------
            lsc_sb = cpool.tile([K, 1], F32)
            nc.sync.dma_start(lsc_sb[:], lsc_d.ap())
            cen_sb = cpool.tile([K, D], F32)
            nc.sync.dma_start(cen_sb[:], cen_d.ap())
            idb_sb = cpool.tile([128, 128], BF16)
            nc.sync.dma_start(idb_sb[:], idb_d.ap())
            idf_sb = cpool.tile([64, 64], F32)
            nc.sync.dma_start(idf_sb[:], idf_d.ap())
            ones_sb = cpool.tile([1, 128], F32)
            nc.sync.dma_start(ones_sb[:], ones_d.ap())
            ctb_sb = cpool.tile([128, ND, K], BF16)
            nc.sync.dma_start(ctb_sb[:], ctb_d.ap().rearrange("(c p) k -> p c k", p=128))

            # big projection weights (phase 2 only) ride the scalar HWDGE FIFO
            wvt_sb = cpool.tile([128, ND, D], BF16)
            nc.scalar.dma_start(wvt_sb[:], wvt_d.ap().rearrange("(c p) e -> p c e", p=128))
            wot_sb = cpool.tile([128, ND, D], BF16)
            nc.scalar.dma_start(wot_sb[:], wot_d.ap().rearrange("(c p) f -> p c f", p=128))

            x2_sb = cpool.tile([128, NCH], F32)      # |x_s|^2 per chunk column
            den_sb = cpool.tile([128, NCH], F32)
            rden_sb = cpool.tile([128, NCH], F32)
            afft_sb = cpool.tile([64, SH], BF16)     # aff.T for own-half chunks
            y_sb = cpool.tile([K, D], F32)
            z_bf = cpool.tile([K, D], BF16)
            w2_sb = cpool.tile([K, D], F32)
            yt_sb = cpool.tile([128, ND, K], BF16)
            w2t_sb = cpool.tile([128, ND, K], BF16)

            # ---- splat constants --------------------------------------------------
            sig = cpool.tile([K, 1], F32)
            nc.scalar.activation(sig[:], lsc_sb[:], mybir.ActivationFunctionType.Exp)
            nc.vector.tensor_scalar_max(sig[:], sig[:], 0.1)
            nc.vector.tensor_scalar_min(sig[:], sig[:], 2.0)
            sig2 = cpool.tile([K, 1], F32)
            nc.vector.tensor_tensor(sig2[:], sig[:], sig[:], mybir.AluOpType.mult)
            inv_col = cpool.tile([K, 1], F32)        # 1/(2 sig^2)
            nc.vector.reciprocal(inv_col[:], sig2[:])
            nc.vector.tensor_scalar_mul(inv_col[:], inv_col[:], 0.5)

            c2scr = cpool.tile([K, D], F32)
            c2_col = cpool.tile([K, 1], F32)
            nc.scalar.activation(
                c2scr[:], cen_sb[:], mybir.ActivationFunctionType.Square,
                accum_out=c2_col[:],
            )

            with tc.tile_pool(name="pscst", bufs=2, space="PSUM") as pscst:
                invrow_ps = pscst.tile([1, 64], F32)
                nc.tensor.transpose(invrow_ps[:], inv_col[:], idf_sb[:])
                invrow_sb = cpool.tile([1, 64], F32)
                nc.vector.tensor_copy(invrow_sb[:], invrow_ps[:])
                invb_ps = pscst.tile([128, 64], F32)
                nc.tensor.matmul(invb_ps[:], ones_sb[:], invrow_sb[:])
                invb_sb = cpool.tile([128, 64], F32)
                nc.vector.tensor_copy(invb_sb[:], invb_ps[:])

            # ---- phase 1: affinities + y = aff.T @ x over the full batch ----------
            with (
                tc.tile_pool(name="psxc", bufs=2, space="PSUM") as psxc,
                tc.tile_pool(name="psbt", bufs=2, space="PSUM") as psbt,
                tc.tile_pool(name="psat", bufs=2, space="PSUM") as psat,
                tc.tile_pool(name="psy", bufs=1, space="PSUM") as psy,
                tc.tile_pool(name="affp", bufs=4) as aff_pool,
            ):
                psum_y = psy.tile([K, D], F32)
                for blk in range(NB):
                    xt_t = xt_pool.tile([128, ND, 512], BF16)
                    nc.sync.dma_start(
                        xt_t[:],
                        xt_d.ap()[:, blk * 512:(blk + 1) * 512]
                        .rearrange("(c p) s -> p c s", p=128),
                    )
                    xn_t = xn_pool.tile([128, 4, D], BF16)
                    nc.sync.dma_start(
                        xn_t[:],
                        xn_d.ap()[blk * 512:(blk + 1) * 512, :]
                        .rearrange("(c p) d -> p c d", p=128),
                    )
                    psum_xc = psxc.tile([K, 512], F32)
                    for dj in range(ND):
                        nc.tensor.matmul(
                            psum_xc[:],
                            ctb_sb[:, dj, :],
                            xt_t[:, dj, :],
                            start=(dj == 0), stop=(dj == ND - 1),
                        )
                    # adj = (2xc - c2) * inv  (k-major layout)
                    adj_sb = adj_pool.tile([K, 512], F32)
                    nc.vector.tensor_scalar(
                        adj_sb[:], psum_xc[:], c2_col[:], inv_col[:],
                        mybir.AluOpType.subtract, mybir.AluOpType.mult,
                    )
                    for j2 in range(4):
                        j = blk * 4 + j2
                        sq = scr_pool.tile([128, D], F32, tag="sq")
                        nc.scalar.activation(
                            sq[:], xn_t[:, j2, :], mybir.ActivationFunctionType.Square,
                            accum_out=x2_sb[:, j:j + 1],
                        )
                        # transpose adj chunk -> [s, k]
                        bt_ps = psbt.tile([128, 64], F32)
                        nc.tensor.transpose(
                            bt_ps[:], adj_sb[:, j2 * 128:(j2 + 1) * 128], idf_sb[:]
                        )
                        # t = inv*x2 - inv*adj = inv * d2
                        t_sb = t_pool.tile([128, 64], F32, tag="t")
                        nc.vector.scalar_tensor_tensor(
                            t_sb[:], invb_sb[:], x2_sb[:, j:j + 1], bt_ps[:],
                            mybir.AluOpType.mult, mybir.AluOpType.subtract,
                        )
                        nc.vector.tensor_scalar_max(t_sb[:], t_sb[:], 0.0)
                        affu = t_pool.tile([128, 64], F32, tag="affu")
                        nc.scalar.activation(
                            affu[:], t_sb[:], mybir.ActivationFunctionType.Exp,
                            scale=-1.0, accum_out=den_sb[:, j:j + 1],
                        )
                        nc.vector.tensor_scalar_add(
                            den_sb[:, j:j + 1], den_sb[:, j:j + 1], 1e-8
                        )
                        nc.vector.reciprocal(rden_sb[:, j:j + 1], den_sb[:, j:j + 1])
                        aff_bf = aff_pool.tile([128, 64], BF16)
                        nc.vector.tensor_scalar_mul(
                            aff_bf[:], affu[:], rden_sb[:, j:j + 1]
                        )
                        # y += aff.T @ x  (accumulated over all chunks)
                        for dh in range(2):
                            nc.tensor.matmul(
                                psum_y[:, dh * 512:(dh + 1) * 512],
                                aff_bf[:],
                                xn_t[:, j2, dh * 512:(dh + 1) * 512],
                                start=(j == 0), stop=(j == NCH - 1),
                            )
                        # aff.T for the output matmul (own half only)
                        if j < NOCH:
                            at_ps = psat.tile([64, 128], BF16)
                            nc.tensor.transpose(at_ps[:], aff_bf[:], idb_sb[:])
                            nc.vector.tensor_copy(
                                afft_sb[:, j * 128:(j + 1) * 128], at_ps[:]
                            )
                nc.vector.tensor_copy(y_sb[:], psum_y[:])

            # ---- phase 2: Z = (y @ Wv.T) @ Wo.T -----------------------------------
            with (
                tc.tile_pool(name="pst2", bufs=2, space="PSUM") as pst2,
                tc.tile_pool(name="psw", bufs=2, space="PSUM") as psw,
            ):
                for dj in range(ND):
                    tr = pst2.tile([128, 64], F32)
                    nc.tensor.transpose(
                        tr[:], y_sb[:, dj * 128:(dj + 1) * 128], idf_sb[:]
                    )
                    nc.vector.tensor_copy(yt_sb[:, dj, :], tr[:])
                for eh in range(2):
                    psum_w2 = psw.tile([K, 512], F32, tag="w2")
                    for dj in range(ND):
                        nc.tensor.matmul(
                            psum_w2[:],
                            yt_sb[:, dj, :],
                            wvt_sb[:, dj, eh * 512:(eh + 1) * 512],
                            start=(dj == 0), stop=(dj == ND - 1),
                        )
                    nc.vector.tensor_copy(w2_sb[:, eh * 512:(eh + 1) * 512], psum_w2[:])
                for ej in range(ND):
                    tr = pst2.tile([128, 64], F32)
                    nc.tensor.transpose(
                        tr[:], w2_sb[:, ej * 128:(ej + 1) * 128], idf_sb[:]
                    )
                    nc.vector.tensor_copy(w2t_sb[:, ej, :], tr[:])
                for fh in range(2):
                    psum_z = psw.tile([K, 512], F32, tag="z")
                    for ej in range(ND):
                        nc.tensor.matmul(
                            psum_z[:],
                            w2t_sb[:, ej, :],
                            wot_sb[:, ej, fh * 512:(fh + 1) * 512],
                            start=(ej == 0), stop=(ej == ND - 1),
                        )
                    nc.vector.tensor_copy(z_bf[:, fh * 512:(fh + 1) * 512], psum_z[:])

            # ---- phase 3: out = aff @ Z (own token half) --------------------------
            with tc.tile_pool(name="pso", bufs=2, space="PSUM") as pso:
                for g in range(NOCH // 4):
                    o_sb = o_pool.tile([128, 4, D], out_dt)
                    for j2 in range(4):
                        j = g * 4 + j2
                        psum_o = pso.tile([128, D], F32)
                        for fh in range(2):
                            nc.tensor.matmul(
                                psum_o[:, fh * 512:(fh + 1) * 512],
                                afft_sb[:, j * 128:(j + 1) * 128],
                                z_bf[:, fh * 512:(fh + 1) * 512],
                                start=True, stop=True,
                            )
                        nc.vector.tensor_copy(o_sb[:, j2, :], psum_o[:])
                    nc.sync.dma_start(
                        out_d.ap()[g * 512:(g + 1) * 512, :]
                        .rearrange("(c p) d -> p c d", p=128),
                        o_sb[:],
                    )

    nc.compile()
    return nc


def _get_nc():
    if "nc" not in _CACHE:
        _CACHE["nc"] = _build_nc()
    return _CACHE["nc"]


def kernel(token_embeddings, splat_centers, splat_log_scales, Wv, Wo):
    x = np.asarray(token_embeddings, dtype=np.float32)
    centers = np.asarray(splat_centers, dtype=np.float32)
    log_scales = np.asarray(splat_log_scales, dtype=np.float32)
    Wv = np.asarray(Wv, dtype=np.float32)
    Wo = np.asarray(Wo, dtype=np.float32)

    nc = _get_nc()

    shared = {
        "ctb": np.ascontiguousarray((2.0 * centers).T).astype(NPBF16),
        "cen": centers,
        "lsc": log_scales.reshape(K, 1),
        "wvt": np.ascontiguousarray(Wv.T).astype(NPBF16),
        "wot": np.ascontiguousarray(Wo.T).astype(NPBF16),
        "idb": np.eye(128, dtype=NPBF16),
        "idf": np.eye(64, dtype=np.float32),
        "ones": np.ones((1, 128), dtype=np.float32),
    }
    in_maps = []
    for b in range(B):
        xb_bf = x[b].astype(NPBF16)                       # [S, D]
        xbt_bf = np.ascontiguousarray(x[b].T).astype(NPBF16)  # [D, S]
        for h in range(2):
            own = slice(h * SH, (h + 1) * SH)
            oth = slice((1 - h) * SH, (2 - h) * SH)
            m = dict(shared)
            m["xn"] = np.concatenate([xb_bf[own], xb_bf[oth]], axis=0)
            m["xt"] = np.concatenate([xbt_bf[:, own], xbt_bf[:, oth]], axis=1)
            in_maps.append(m)

    res = bass_utils.run_bass_kernel_spmd(nc, in_maps, core_ids=list(range(NCORES)))

    out = np.empty((B, S, D), dtype=np.float32)
    for c in range(NCORES):
        b, h = divmod(c, 2)
        out[b, h * SH:(h + 1) * SH] = res.results[c]["out"].astype(np.float32)
    return out
